# revision 1
# baseline (speedup 1.0000x reference)
"""MoE MLP (top-1 routing) on 8 TRN2 NeuronCores.

Strategy (expert-parallel, per the sharding hint): the host computes the
router argmax and dispatches each token to its expert's core. Core e holds
w_fc[e]/w_proj[e] and runs the dense expert MLP
    y = (0.5*(x_fc + relu(x_fc)))^2 @ w_proj[e].T,   x_fc = x @ w_fc[e].T
on its gathered tokens in a transposed (feature-major) layout so no on-device
transposes are needed. Matmuls run as float32r (full-rate fp32 mode on the PE).

Tokens are processed in near-equal blocks of <=512. The x and y DRAM layouts
are block-major so each block transfer is one fully-contiguous-per-partition
DMA. The emission order software-pipelines blocks on the PE: GEMM2 of block
b-1 is issued between GEMM1 of block b and its activation epilogue, so the PE
never waits on the ACT/DVE chain.
"""

import contextlib
import numpy as np

import concourse.mybir as mybir
import concourse.tile as tile
from concourse import bacc
from concourse.bass_utils import run_bass_kernel_spmd

P = 128          # SBUF partitions / PE array dim
D = 1024         # model dim
E = 8            # experts == cores
H = 512          # expert hidden dim
KD = D // P      # k-chunks over D
KH = H // P      # k-chunks over H
MD = D // P      # output d-tiles
TB = 512         # max token block (PSUM bank / fp32 moving-operand limit)

F32 = mybir.dt.float32
F32R = mybir.dt.float32r
AF = mybir.ActivationFunctionType
ALU = mybir.AluOpType

_programs = {}
last_exec_ns = None


def _token_blocks(C):
    # Near-equal blocks of at most TB tokens (multiples of 128). Keeping
    # every block >= 256 tokens holds fp32r matmuls at full rate.
    chunks = C // P
    nb = -(-chunks // (TB // P))
    q, r = divmod(chunks, nb)
    sizes = [(q + 1) * P] * r + [q * P] * (nb - r)
    blocks = []
    t = 0
    for tb in sizes:
        blocks.append((t, tb))
        t += tb
    return blocks


def _build_program(C, repeat=1, sim_safe=False, w_dtype=F32R):
    nc = bacc.Bacc("TRN2", target_bir_lowering=False, debug=False)
    xk = nc.declare_dram_parameter("xk", [P, KD * C], F32R, isOutput=False)
    wfck = nc.declare_dram_parameter("wfck", [P, KD, H], w_dtype, isOutput=False)
    wpjk = nc.declare_dram_parameter("wpjk", [P, KH, D], w_dtype, isOutput=False)
    yk = nc.declare_dram_parameter("yk", [P, C * MD], F32, isOutput=True)

    blocks = _token_blocks(C)

    with tile.TileContext(nc) as tc:
        with (
            tc.tile_pool(name="wpool", bufs=1) as wpool,
            tc.tile_pool(name="xpool", bufs=3) as xpool,
            tc.tile_pool(name="hpool", bufs=2) as hpool,
            tc.tile_pool(name="ypool", bufs=2) as ypool,
            tc.tile_pool(name="spool", bufs=3) as spool,
            tc.tile_pool(name="hpsum", bufs=3, space="PSUM") as hpsum,
            tc.tile_pool(name="ypsum", bufs=4, space="PSUM") as ypsum,
            contextlib.ExitStack() as loop_ctx,
        ):
            wfc_sb = wpool.tile([P, KD, H], w_dtype)
            nc.sync.dma_start(wfc_sb[:], wfck[:])
            wpj_sb = wpool.tile([P, KH, D], w_dtype)
            nc.sync.dma_start(wpj_sb[:], wpjk[:])

            if repeat > 1:
                loop_ctx.enter_context(
                    tc.For_i(0, repeat, 1,
                             hint_engines=(mybir.EngineType.PE,)))

            def g1_group(x_sb, h_sb, tb, m):
                ph = hpsum.tile([P, tb], F32, tag="ph")
                for k in range(KD):
                    nc.tensor.matmul(
                        ph[:],
                        wfc_sb[:, k, m * P:(m + 1) * P],
                        x_sb[:, k, :],
                        start=(k == 0),
                        stop=(k == KD - 1),
                    )
                # h = leaky_relu_0.5(ph)^2 = (0.5*(relu(ph) + ph))^2
                # (ACT Lrelu's alpha operand produces wrong results on HW,
                # so use this 3-op form: Relu -> fused add -> Square.)
                r_sb = spool.tile([P, tb], F32, tag="r")
                nc.scalar.activation(r_sb[:], ph[:], AF.Relu)
                s_sb = spool.tile([P, tb], F32, tag="s")
                nc.vector.scalar_tensor_tensor(
                    s_sb[:], r_sb[:], 0.0, ph[:], ALU.add, ALU.add)
                nc.scalar.activation(
                    h_sb[:, m, :], s_sb[:], AF.Square, scale=0.5)

            def g2_group(h_sb, y_blk, tb, j):
                py = ypsum.tile([P, tb], F32, tag="py")
                for kh in range(KH):
                    nc.tensor.matmul(
                        py[:],
                        wpj_sb[:, kh, j * P:(j + 1) * P],
                        h_sb[:, kh, :],
                        start=(kh == 0),
                        stop=(kh == KH - 1),
                    )
                nc.vector.tensor_copy(y_blk[:, :, j], py[:])

            def y_store(t0, tb, y_blk):
                dst = yk[:, MD * t0:MD * (t0 + tb)].rearrange(
                    "p (t j) -> p t j", j=MD)
                nc.sync.dma_start(dst, y_blk[:])

            # Software pipeline with fine interleave: between the m-groups
            # of GEMM1(b), emit the j-groups of GEMM2(b-1), so the PE always
            # has independent work while block b's epilogue runs.
            prev = None
            for (t0, tb) in blocks:
                x_sb = xpool.tile([P, KD, tb], F32R, tag="x")
                src = xk[:, KD * t0:KD * (t0 + tb)].rearrange(
                    "p (k t) -> p k t", k=KD)
                nc.sync.dma_start(x_sb[:], src)
                h_sb = hpool.tile([P, KH, tb], F32R, tag="h")
                if prev is not None:
                    p0, ptb, ph_sb = prev
                    y_blk = ypool.tile([P, ptb, MD], F32, tag="y")
                for m in range(KH):
                    g1_group(x_sb, h_sb, tb, m)
                    if prev is not None:
                        g2_group(ph_sb, y_blk, ptb, 2 * m)
                        g2_group(ph_sb, y_blk, ptb, 2 * m + 1)
                if prev is not None:
                    y_store(p0, ptb, y_blk)
                prev = (t0, tb, h_sb)
            p0, ptb, ph_sb = prev
            y_blk = ypool.tile([P, ptb, MD], F32, tag="y")
            for j in range(MD):
                g2_group(ph_sb, y_blk, ptb, j)
            y_store(p0, ptb, y_blk)

    nc.compile()
    return nc


def _program(C):
    if C not in _programs:
        _programs[C] = _build_program(C)
    return _programs[C]


def _pack_x(xg, C):
    """[C, D] tokens -> block-major [P, KD*C] f32 array."""
    parts = []
    for (t0, tb) in _token_blocks(C):
        blk = xg[t0:t0 + tb].reshape(tb, KD, P).transpose(2, 1, 0)
        parts.append(blk.reshape(P, KD * tb))
    return np.ascontiguousarray(np.concatenate(parts, axis=1))


def _pack_wfc(wfc_e):
    return np.ascontiguousarray(wfc_e.T.reshape(KD, P, H).transpose(1, 0, 2))


def _pack_wproj(wproj_e):
    return np.ascontiguousarray(wproj_e.T.reshape(KH, P, D).transpose(1, 0, 2))


def _unpack_y(yk_arr, C):
    """[P, C*MD] -> [C, D]."""
    return yk_arr.reshape(P, C, MD).transpose(1, 2, 0).reshape(C, D)


def kernel(x, w_router, w_fc, w_proj):
    global last_exec_ns
    x = np.asarray(x, dtype=np.float32)
    w_router = np.asarray(w_router, dtype=np.float32)
    w_fc = np.asarray(w_fc, dtype=np.float32)
    w_proj = np.asarray(w_proj, dtype=np.float32)

    B, S, _ = x.shape
    N = B * S
    xf = np.ascontiguousarray(x.reshape(N, D))

    # Host-side router: top-1 expert per token (softmax is monotone, so
    # argmax over logits == argmax over softmax weights).
    logits = xf @ w_router.T
    eidx = np.argmax(logits, axis=1)
    counts = np.bincount(eidx, minlength=E)
    order = np.argsort(eidx, kind="stable")
    offs = np.concatenate(([0], np.cumsum(counts)))

    C = max(P, -(-int(counts.max()) // P) * P)  # round up to 128

    in_maps = []
    tok_ids = []
    for e in range(E):
        ids = order[offs[e]:offs[e + 1]]
        tok_ids.append(ids)
        xg = np.zeros((C, D), np.float32)
        xg[:len(ids)] = xf[ids]
        in_maps.append({
            "xk": _pack_x(xg, C),
            "wfck": _pack_wfc(w_fc[e]),
            "wpjk": _pack_wproj(w_proj[e]),
        })

    nc = _program(C)
    res = run_bass_kernel_spmd(nc, in_maps, core_ids=list(range(E)))
    last_exec_ns = res.exec_time_ns

    out = np.zeros((N, D), np.float32)
    for e in range(E):
        yg = _unpack_y(np.asarray(res.results[e]["yk"]), C)
        out[tok_ids[e]] = yg[:counts[e]]
    return out.reshape(B, S, D)

